# revision 1
# baseline (speedup 1.0000x reference)
"""AttentionPool2d Trainium2 kernel, 8-core batch-data-parallel.

Math (reference returns only query position 0):
  xf = [x.flat, mean] + pos  (permuted: cols 0..255 spatial, col 256 = mean tok)
  q0 = W_q @ xf_m + b_q                 (the only query needed)
  u_h = W_k_h^T q0_h  (folds W_k into the query; k never materialized)
  logits = (1/8) u^T xf ; w = softmax(logits)
  y = xf @ w'^T (+ pos-terms)           (w' = w_sp + w_m/256 absorbs mean token)
  a0_h = W_v_h y_h + b_v ; out = w_c a0 + b_c
"""
import sys, types
sys.path.insert(0, "/opt/trn_rl_repo")
import numpy as np
import ml_dtypes
from contextlib import ExitStack

from concourse import bacc, tile, mybir
import concourse.bass as bass
from concourse import masks
from concourse.bass_utils import run_bass_kernel_spmd

P = 128
B, C, S2, L = 64, 1024, 256, 257
NH, CHD = 16, 64
NCORE, BPC, CT = 8, 8, 8          # cores, batches/core, c-tiles
F32R = mybir.dt.float32r
F32 = mybir.dt.float32
BF16 = mybir.dt.bfloat16
AF = mybir.ActivationFunctionType
SCALE2 = 1.0 / 8.0                 # (1/ch^0.25)^2 folded into u


def _body(ctx: ExitStack, tc, d):
    nc = tc.nc
    const = ctx.enter_context(tc.tile_pool(name="const", bufs=1))
    wbig = ctx.enter_context(tc.tile_pool(name="wbig", bufs=2))
    wsml = ctx.enter_context(tc.tile_pool(name="wsml", bufs=1))
    xres = ctx.enter_context(tc.tile_pool(name="xres", bufs=1))
    xtp = ctx.enter_context(tc.tile_pool(name="xtp", bufs=1))
    wbf = ctx.enter_context(tc.tile_pool(name="wbf", bufs=2))
    work = ctx.enter_context(tc.tile_pool(name="work", bufs=1))
    acc = ctx.enter_context(tc.tile_pool(name="acc", bufs=1))
    ps = ctx.enter_context(tc.tile_pool(name="ps", bufs=2, space="PSUM"))
    ps1 = ctx.enter_context(tc.tile_pool(name="ps1", bufs=2, space="PSUM"))

    identf = const.tile([16, 16], F32)
    masks.make_identity(nc, identf[:])
    ident = const.tile([16, 16], F32R)
    nc.vector.tensor_copy(ident[:], identf[:, :])

    # ---- stage A: x in, means, xf0 ----
    xs = []
    sums = acc.tile([P, BPC * CT], F32R)
    xf0 = acc.tile([P, BPC * CT], BF16)             # mean-token cols (b, j)
    scratch = work.tile([P, S2], F32R, tag="scr")
    xpairs = []
    for pr in range(BPC // 2):
        xp2 = xres.tile([P, 2, CT, S2 + 2], BF16, tag=f"xp{pr}")
        nc.sync.dma_start(
            xp2[:, :, :, 0:S2],
            d["x"].ap()[2 * pr:2 * pr + 2].rearrange(
                "b (j p) s -> p (b j) s", p=P).rearrange(
                "p (b j) s -> p b j s", b=2))
        nc.vector.tensor_scalar_mul(xp2[:, :, :, S2 + 1:S2 + 2],
                                     xp2[:, :, :, 0:1], 0.0)
        xpairs.append(xp2)
    for b in range(BPC):
        xb = xpairs[b // 2][:, b % 2]
        xs.append(xb)

    # ---- weights needed early (after x DMAs in queue order) ----
    wqt = wbig.tile([P, CT, C], BF16, tag="wbig")   # W_q^T  (c-part, q)
    nc.sync.dma_start(wqt[:], d["wqt"].ap().rearrange("(j p) q -> p j q", p=P))
    wk = wbig.tile([P, CT, C], BF16, tag="wbig")    # W_k natural (krow-part, c)
    nc.sync.dma_start(wk[:], d["wk"].ap().rearrange("(t k) c -> k t c", k=P))
    posn = wsml.tile([P, CT, L], BF16)              # permuted pos, natural
    nc.sync.dma_start(posn[:], d["posn"].ap().rearrange("(j p) s -> p j s", p=P))
    post = wsml.tile([P, 2, C], BF16)               # spatial pos, transposed
    nc.sync.dma_start(post[:], d["post"].ap().rearrange("(t p) c -> p t c", p=P))
    posc = wsml.tile([1, C], BF16)                  # pos0 - mean_sp(pos)
    nc.sync.dma_start(posc[:], d["posc"].ap())
    bq = wsml.tile([P, CT], F32R)
    nc.sync.dma_start(bq[:], d["bq"].ap().rearrange("(j p) -> p j", p=P))
    bv = wsml.tile([P, CT], F32R)
    nc.sync.dma_start(bv[:], d["bv"].ap().rearrange("(j p) -> p j", p=P))
    bcn = wsml.tile([P, CT], F32R)
    nc.sync.dma_start(bcn[:], d["bc"].ap().rearrange("(j p) -> p j", p=P))
    wvt = wbf.tile([P, CT, C], BF16, tag="wv")      # W_v^T (c-part, vch)
    nc.sync.dma_start(wvt[:], d["wvt"].ap().rearrange("(j p) v -> p j v", p=P))
    wct = wbf.tile([P, CT, C], BF16, tag="wc")      # w_c^T (vch-part, o)
    nc.sync.dma_start(wct[:], d["wct"].ap().rearrange("(r p) o -> p r o", p=P))

    for b in range(BPC):
        xb = xs[b]
        for j in range(CT):
            if j % 2 == 0:
                nc.vector.reduce_sum(sums[:, b * CT + j:b * CT + j + 1],
                                     xb[:, j, 0:S2], axis=mybir.AxisListType.X)
            else:
                nc.scalar.activation(scratch[:], xb[:, j, 0:S2], AF.Copy,
                                     accum_out=sums[:, b * CT + j:b * CT + j + 1])
        for j in range(CT):
            nc.scalar.activation(xf0[:, b * CT + j:b * CT + j + 1],
                                 sums[:, b * CT + j:b * CT + j + 1], AF.Identity,
                                 bias=posn[:, j, S2:S2 + 1], scale=1.0 / S2)
            nc.scalar.activation(xb[:, j, S2:S2 + 1],
                                 sums[:, b * CT + j:b * CT + j + 1], AF.Identity,
                                 bias=posn[:, j, S2:S2 + 1], scale=1.0 / S2)

    # ---- stage B: q0 (batched over b) ----
    q0f = ps1.tile([P, P], F32, tag="seq")
    q0p = q0f[:, 0:CT * BPC]        # (q-part, (i, b))
    for i in range(CT):
        for j in range(CT):
            nc.tensor.matmul(q0p[:, i * BPC:(i + 1) * BPC],
                             wqt[:, j, i * P:(i + 1) * P],
                             xf0[:, b0j(j)],
                             start=(j == 0), stop=(j == CT - 1))
    # block-diagonal q0 (+bias) for the per-head W_k^T fold
    q0blk = acc.tile([P, CT * 16], BF16)
    nc.vector.memset(q0blk[:], 0.0)
    for i in range(CT):
        nc.scalar.activation(q0blk[0:64, i * 16:i * 16 + 8],
                             q0p[0:64, i * BPC:i * BPC + 8], AF.Identity,
                             bias=bq[0:64, i:i + 1])
        nc.scalar.activation(q0blk[64:P, i * 16 + 8:i * 16 + 16],
                             q0p[64:P, i * BPC:i * BPC + 8], AF.Identity,
                             bias=bq[64:P, i:i + 1])

    # ---- stage C: u = blockdiag(W_k)^T q0, scaled ----
    usb = acc.tile([P, CT * P], BF16)               # (c-part, (j, h, b))
    for j in range(CT):
        up = ps1.tile([P, P], F32, tag="seq")
        for t in range(CT):
            nc.tensor.matmul(up[:, t * 16:(t + 1) * 16],
                             wk[:, t, j * P:(j + 1) * P],
                             q0blk[:, t * 16:(t + 1) * 16])
        nc.vector.tensor_scalar_mul(usb[:, j * P:(j + 1) * P], up[:, :], SCALE2)

    # ---- per-batch: logits, softmax, w' transposes, y_x ----
    xtall = xtp.tile([P, 2 * BPC, C], BF16)
    nc.sync.dma_start(xtall[:], d["xt"].ap().rearrange(
        "b (t p) c -> p (b t) c", p=P))
    wta = acc.tile([P, 3 * P], BF16)                # w'^T batched (s-part,(t,h,b))
    yall = acc.tile([P, CT * P], BF16)              # y (c-part, (j, h, b))
    ypsb = acc.tile([P, CT * P], BF16)              # y_pos (c-part, (j, h, b))
    for b in range(BPC):
        lg = ps.tile([16, S2 + 2], F32, tag="lg")
        ub = [usb[:, j * P + b: (j + 1) * P: 8] for j in range(CT)]
        for j in range(CT):
            nc.tensor.matmul(lg[:, 0:S2 + 2], ub[j], xs[b][:, j, :],
                             start=(j == 0), stop=False)
        for j in range(CT):
            nc.tensor.matmul(lg[:, 0:S2], ub[j], posn[:, j, 0:S2],
                             start=False, stop=(j == CT - 1))
        # softmax over 257
        mx = work.tile([16, 4], F32, tag="mx")
        nc.vector.reduce_max(mx[:, 0:1], lg[:, 0:L], axis=mybir.AxisListType.X,
                             negate=True)
        ex = work.tile([16, L], F32R, tag="ex")
        nc.scalar.activation(ex[:, :], lg[:, 0:L], AF.Exp, bias=mx[:, 0:1],
                             accum_out=mx[:, 1:2])
        nc.vector.reciprocal(mx[:, 2:3], mx[:, 1:2])
        # w' = (e_sp + e_m/256) * r ; wm = e_m * r
        wp = work.tile([16, L], F32R, tag="wp")
        nc.vector.tensor_scalar_mul(mx[:, 3:4], ex[:, S2:S2 + 1], 1.0 / S2)
        nc.vector.tensor_scalar(wp[:, 0:S2], ex[:, 0:S2], mx[:, 3:4], mx[:, 2:3],
                                op0=mybir.AluOpType.add,
                                op1=mybir.AluOpType.mult)
        nc.vector.tensor_scalar(wp[:, S2:L], ex[:, S2:L], mx[:, 2:3], None,
                                op0=mybir.AluOpType.mult)
        # transpose w' -> (s-part, h) chunks; third chunk = wm row
        wtp = ps.tile([P, 48], F32R, tag="wt")
        nc.tensor.transpose(wtp[:, 0:16], wp[:, 0:P],
                            ident[:, :])
        nc.tensor.transpose(wtp[:, 16:32], wp[:, P:S2],
                            ident[:, :])
        nc.tensor.transpose(wtp[0:1, 32:48], wp[:, S2:L],
                            ident[:, :])
        for t in range(2):
            nc.vector.tensor_copy(wta[:, t * P + b:(t + 1) * P:8],
                                  wtp[:, t * 16:(t + 1) * 16])
        nc.vector.tensor_copy(wta[0:1, 2 * P + b:3 * P:8], wtp[0:1, 32:48])
        # y_x: stationary x^T tiles, moving w'^T
        yp = ps.tile([P, P], F32, tag="y")
        for j in range(CT):
            for t in range(2):
                nc.tensor.matmul(yp[:, j * 16:(j + 1) * 16],
                                 xtall[:, 2 * b + t, j * P:(j + 1) * P],
                                 wta[:, t * P + b:(t + 1) * P:8],
                                 start=(t == 0), stop=(t == 1))
        # scatter y_b into (j, h, b) layout: stride-8 columns for batch b
        nc.vector.tensor_copy(yall[:, b::8], yp[:, :])

    # ---- y_pos batched: pos^T against all-b w'^T ----
    for j in range(CT):
        ypp = ps1.tile([P, P], F32, tag="seq")
        for t in range(2):
            nc.tensor.matmul(ypp[:, :], post[:, t, j * P:(j + 1) * P],
                             wta[:, t * P:(t + 1) * P], start=(t == 0), stop=False)
        nc.tensor.matmul(ypp[:, :], posc[0:1, j * P:(j + 1) * P],
                         wta[0:1, 2 * P:3 * P], start=False, stop=True)
        nc.vector.tensor_copy(ypsb[:, j * P:(j + 1) * P], ypp[:, :])
    yfin = acc.tile([P, CT * P], BF16)
    nc.vector.tensor_add(yfin[:, :], yall[:, :], ypsb[:, :])

    # ---- a0 = blockdiag(W_v) y  (+ b_v) ----
    a0p = ps1.tile([P, P], F32, tag="seq")
    for r in range(CT):
        for j in range(CT):
            nc.tensor.matmul(a0p[:, r * 16:(r + 1) * 16],
                             wvt[:, j, r * P:(r + 1) * P],
                             yfin[:, j * P + 2 * r * 8: j * P + 2 * r * 8 + 16],
                             start=(j == 0), stop=(j == CT - 1))
    a0 = acc.tile([P, CT * BPC], BF16)              # (vch-part, (r, b))
    for r in range(CT):
        nc.scalar.activation(a0[0:64, r * 8:(r + 1) * 8],
                             a0p[0:64, r * 16:r * 16 + 8], AF.Identity,
                             bias=bv[0:64, r:r + 1])
        nc.scalar.activation(a0[64:P, r * 8:(r + 1) * 8],
                             a0p[64:P, r * 16 + 8:(r + 1) * 16], AF.Identity,
                             bias=bv[64:P, r:r + 1])

    # ---- out = w_c a0 + b_c ----
    opf = ps1.tile([P, P], F32, tag="seq")
    op = opf[:, 0:CT * BPC]
    for i in range(CT):
        for r in range(CT):
            nc.tensor.matmul(op[:, i * BPC:(i + 1) * BPC],
                             wct[:, r, i * P:(i + 1) * P],
                             a0[:, r * BPC:(r + 1) * BPC],
                             start=(r == 0), stop=(r == CT - 1))
    osb = acc.tile([P, CT * BPC], F32)
    for i in range(CT):
        nc.scalar.activation(osb[:, i * BPC:(i + 1) * BPC],
                             op[:, i * BPC:(i + 1) * BPC], AF.Identity,
                             bias=bcn[:, i:i + 1])
    nc.sync.dma_start(d["out"].ap(), osb[:])


def b0j(j):
    # xf0 columns for all b at fixed j: (b, j) layout -> stride CT
    return slice(j, BPC * CT, CT)


_CACHE = {}


def _get_nc():
    if "nc" in _CACHE:
        return _CACHE["nc"]
    nc = bacc.Bacc("TRN2", target_bir_lowering=False, debug=False,
                   num_devices=NCORE)
    d = {}
    d["x"] = nc.dram_tensor("x", [BPC, C, S2], BF16, kind="ExternalInput")
    d["xt"] = nc.dram_tensor("xt", [BPC, S2, C], BF16, kind="ExternalInput")
    d["posn"] = nc.dram_tensor("posn", [C, L], BF16, kind="ExternalInput")
    d["post"] = nc.dram_tensor("post", [S2, C], BF16, kind="ExternalInput")
    d["posc"] = nc.dram_tensor("posc", [1, C], BF16, kind="ExternalInput")
    d["wqt"] = nc.dram_tensor("wqt", [C, C], BF16, kind="ExternalInput")
    d["wk"] = nc.dram_tensor("wk", [C, C], BF16, kind="ExternalInput")
    d["wvt"] = nc.dram_tensor("wvt", [C, C], BF16, kind="ExternalInput")
    d["wct"] = nc.dram_tensor("wct", [C, C], BF16, kind="ExternalInput")
    d["bq"] = nc.dram_tensor("bq", [C], F32R, kind="ExternalInput")
    d["bv"] = nc.dram_tensor("bv", [C], F32R, kind="ExternalInput")
    d["bc"] = nc.dram_tensor("bc", [C], F32R, kind="ExternalInput")
    d["out"] = nc.dram_tensor("out", [P, CT * BPC], F32, kind="ExternalOutput")
    with tile.TileContext(nc) as tc, ExitStack() as ctx, \
            nc.allow_low_precision(reason="float32r tiles hold f32 bits"):
        _body(ctx, tc, d)
    nc.compile()
    _CACHE["nc"] = nc
    return nc


def _prep_maps(inputs):
    xf32 = inputs["x"].reshape(B, C, S2).astype(np.float32)
    x = np.ascontiguousarray(xf32).astype(ml_dtypes.bfloat16)
    xt = np.ascontiguousarray(xf32.transpose(0, 2, 1)).astype(ml_dtypes.bfloat16)
    pos = inputs["pos_emb"].astype(np.float32)
    posn = np.ascontiguousarray(np.concatenate([pos[:, 1:], pos[:, :1]], axis=1)).astype(ml_dtypes.bfloat16)
    post = np.ascontiguousarray(pos[:, 1:].T).astype(ml_dtypes.bfloat16)
    posc = np.ascontiguousarray((pos[:, 0] - pos[:, 1:].mean(axis=1))[None, :]
                                ).astype(ml_dtypes.bfloat16)
    wqkv = inputs["w_qkv"].astype(np.float32)
    wqt = np.ascontiguousarray(wqkv[0:C].T).astype(ml_dtypes.bfloat16)
    wk = np.ascontiguousarray(wqkv[C:2 * C]).astype(ml_dtypes.bfloat16)
    wvt = np.ascontiguousarray(wqkv[2 * C:3 * C].T).astype(ml_dtypes.bfloat16)
    wct = np.ascontiguousarray(inputs["w_c"].astype(np.float32).T).astype(ml_dtypes.bfloat16)
    bqkv = inputs["b_qkv"].astype(np.float32)
    shared = dict(posn=posn, post=post, posc=posc, wqt=wqt, wk=wk, wvt=wvt,
                  wct=wct, bq=np.ascontiguousarray(bqkv[0:C]),
                  bv=np.ascontiguousarray(bqkv[2 * C:3 * C]),
                  bc=inputs["b_c"].astype(np.float32))
    maps = []
    for c in range(NCORE):
        m = dict(shared)
        m["x"] = np.ascontiguousarray(x[c * BPC:(c + 1) * BPC])
        m["xt"] = np.ascontiguousarray(xt[c * BPC:(c + 1) * BPC])
        maps.append(m)
    return maps


def kernel(**inputs) -> np.ndarray:
    nc = _get_nc()
    maps = _prep_maps(inputs)
    res = run_bass_kernel_spmd(nc, maps, list(range(NCORE)))
    outs = []
    for c in range(NCORE):
        arr = res.results[c]["out"].reshape(P, CT, BPC)
        outs.append(arr.transpose(2, 1, 0).reshape(BPC, C))
    return np.concatenate(outs, axis=0).astype(np.float32)


if __name__ == "__main__":
    rng = np.random.default_rng(0)
    ins = {
        "x": rng.standard_normal((B, C, 16, 16), dtype=np.float32),
        "pos_emb": rng.standard_normal((C, L), dtype=np.float32) / 32,
        "w_qkv": rng.standard_normal((3 * C, C), dtype=np.float32) / 32,
        "b_qkv": rng.standard_normal((3 * C,), dtype=np.float32) * 0.1,
        "w_c": rng.standard_normal((C, C), dtype=np.float32) / 32,
        "b_c": rng.standard_normal((C,), dtype=np.float32) * 0.1,
    }
    o = kernel(**ins)
    print("out", o.shape, o.dtype, float(np.abs(o).mean()))



# revision 14
# speedup vs baseline: 1.2488x; 1.2488x over previous
"""AttentionPool2d Trainium2 kernel, 8-core batch-data-parallel.

Math (reference returns only query position 0):
  x' = x.flat + pos_sp (pre-added on host); posc = pos_m - mean(pos_sp)
  xf0 = mean_s(x') + posc                   (mean-token input vector)
  q0 = W_q @ xf0 + b_q                      (the only query needed)
  u_h = W_k_h^T q0_h  (folds W_k into the query; k never materialized)
  l_s = (1/8) u^T x'_s ; l_m = mean_s(l_s) + (1/8) u^T posc
  w = softmax(l) ; w' = w_sp + w_m/256      (absorbs mean token)
  y = x'^T w' + w_m * posc
  a0_h = W_v_h y_h + b_v ; out = w_c a0 + b_c

DMA strategy: every input is host-packed so each partition line is one
contiguous >=512B descriptor.  Stream order x' -> W_q -> W_k -> xt' ->
W_v -> W_c with compute chasing the stream (means/q0/u consume slabs as
they land; per-batch attention overlaps the xt' stream; a0/out chase the
tail weights).
"""
import sys
sys.path.insert(0, "/opt/trn_rl_repo")
import numpy as np
import ml_dtypes
from contextlib import ExitStack

from concourse import bacc, tile, mybir
from concourse import masks
from concourse.bass_utils import run_bass_kernel_spmd

P = 128
B, C, S2, L = 64, 1024, 256, 257
NH, CHD = 16, 64
NCORE, BPC, CT = 8, 8, 8          # cores, batches/core, c-tiles
F32R = mybir.dt.float32r
F32 = mybir.dt.float32
BF16 = mybir.dt.bfloat16
AF = mybir.ActivationFunctionType
SCALE2 = 1.0 / 8.0                 # (1/ch^0.25)^2 folded into u


def _body(ctx: ExitStack, tc, d):
    nc = tc.nc
    const = ctx.enter_context(tc.tile_pool(name="const", bufs=1))
    xres = ctx.enter_context(tc.tile_pool(name="xres", bufs=1))
    xtp = ctx.enter_context(tc.tile_pool(name="xtp", bufs=1))
    wts = ctx.enter_context(tc.tile_pool(name="wts", bufs=1))
    work = ctx.enter_context(tc.tile_pool(name="work", bufs=2))
    acc = ctx.enter_context(tc.tile_pool(name="acc", bufs=1))
    ps = ctx.enter_context(tc.tile_pool(name="ps", bufs=2, space="PSUM"))
    ps1 = ctx.enter_context(tc.tile_pool(name="ps1", bufs=2, space="PSUM"))

    identf = const.tile([16, 16], F32)
    masks.make_identity(nc, identf[:])
    ident = const.tile([16, 16], F32R)
    nc.vector.tensor_copy(ident[:], identf[:, :])

    # ---- DMA issue order = stream order ----
    xt = xres.tile([P, CT, BPC, L], BF16)          # x' (c-part): [p, j, b, 257]
    for j in range(CT):
        nc.sync.dma_start(xt[:, j], d["xall"].ap()[j])
    small = wts.tile([P, 4, CT], F32)              # posc, bq, bv, bc (c-part)
    nc.sync.dma_start(small[:], d["small"].ap())
    wvposc = wts.tile([1, C], BF16)                # W_v @ posc, 1-partition
    nc.sync.dma_start(wvposc[:], d["wvposc"].ap())
    wqt = wts.tile([P, CT, C], BF16)               # W_q^T  (c-part, q)
    for h in range(4):
        nc.sync.dma_start(wqt[:, 2 * h:2 * h + 2],
                          d["wqt"].ap()[2 * h:2 * h + 2].rearrange(
                              "j p q -> p j q"))
    wk = wts.tile([P, CT, CT, P], BF16)            # W_k  [kp, j, t, ci]
    for h in range(4):
        nc.sync.dma_start(wk[:, 2 * h:2 * h + 2],
                          d["wk"].ap()[2 * h:2 * h + 2].rearrange(
                              "j p t c -> p j t c"))
    xtt = xtp.tile([P, BPC, 2, C], BF16)           # xt' (s-part): [p, b, t, c]
    for b in range(BPC):
        nc.sync.dma_start(xtt[:, b], d["xtp"].ap()[b])
    wvt = wts.tile([P, CT, C], BF16)               # W_v^T (c-part, vch)
    for h in range(2):
        nc.sync.dma_start(wvt[:, 4 * h:4 * h + 4],
                          d["wvt"].ap()[4 * h:4 * h + 4].rearrange(
                              "j p q -> p j q"))
    wct = wts.tile([P, CT, C], BF16)               # w_c^T (vch-part, o)
    for h in range(2):
        nc.sync.dma_start(wct[:, 4 * h:4 * h + 4],
                          d["wct"].ap()[4 * h:4 * h + 4].rearrange(
                              "j p q -> p j q"))

    # ---- stage A: means chase x' slabs; xf0 = mean + posc ----
    sums = acc.tile([P, CT, BPC], F32R)
    xf0 = acc.tile([P, CT * BPC], BF16)            # (c-part, (j, b))
    for j in range(CT):
        nc.vector.reduce_sum(sums[:, j], xt[:, j, :, 0:S2],
                             axis=mybir.AxisListType.X)
        nc.vector.tensor_scalar(xf0[:, j * BPC:(j + 1) * BPC], sums[:, j],
                                1.0 / S2, small[:, 0, j:j + 1],
                                op0=mybir.AluOpType.mult,
                                op1=mybir.AluOpType.add)

    # ---- stage B: q0 (chases wqt slabs) ----
    q0f = ps1.tile([P, P], F32, tag="seq")
    q0p = q0f[:, 0:CT * BPC]        # (q-part, (i, b))
    for i in range(CT):
        for j in range(CT):
            nc.tensor.matmul(q0p[:, i * BPC:(i + 1) * BPC],
                             wqt[:, j, i * P:(i + 1) * P],
                             xf0[:, j * BPC:(j + 1) * BPC],
                             start=(j == 0), stop=(j == CT - 1))
    # block-diagonal q0 (+bias) for the per-head W_k^T fold
    q0blk = acc.tile([P, CT * 16], BF16)
    nc.vector.memset(q0blk[:], 0.0)
    for i in range(CT):
        nc.scalar.activation(q0blk[0:64, i * 16:i * 16 + 8],
                             q0p[0:64, i * BPC:i * BPC + 8], AF.Identity,
                             bias=small[0:64, 1, i:i + 1])
        nc.scalar.activation(q0blk[64:P, i * 16 + 8:i * 16 + 16],
                             q0p[64:P, i * BPC:i * BPC + 8], AF.Identity,
                             bias=small[64:P, 1, i:i + 1])

    # ---- stage C: u = blockdiag(W_k)^T q0, scaled (chases wk slabs) ----
    usb = acc.tile([P, CT * P], BF16)               # (c-part, (j, t2h, b))
    for j in range(CT):
        up = ps1.tile([P, P], F32, tag="seq")
        for t in range(CT):
            nc.tensor.matmul(up[:, t * 16:(t + 1) * 16],
                             wk[:, j, t],
                             q0blk[:, t * 16:(t + 1) * 16])
        nc.vector.tensor_scalar_mul(usb[:, j * P:(j + 1) * P], up[:, :], SCALE2)

    # ---- per-batch: logits (257-wide, posc col included), softmax, y ----
    wta = acc.tile([P, 3 * P], BF16)                # w'^T batched (s-part,(t,h,b))
    yall = acc.tile([P, CT * P], BF16)              # y (c-part, (j, h, b))
    for b in range(BPC):
        lg = ps.tile([16, L], F32, tag="lg")
        for j in range(CT):
            nc.tensor.matmul(lg[:, 0:L],
                             usb[:, j * P + b: (j + 1) * P: 8],
                             xt[:, j, b, :],
                             start=(j == 0), stop=(j == CT - 1))
        # mean-token logit: l_m = mean_s(l_s) + u^T posc (the latter in col 256)
        mx = work.tile([16, 6], F32, tag="mx")
        nc.vector.reduce_sum(mx[:, 4:5], lg[:, 0:S2], axis=mybir.AxisListType.X)
        nc.vector.tensor_scalar_mul(mx[:, 5:6], mx[:, 4:5], 1.0 / S2)
        nc.vector.tensor_scalar_add(lg[:, S2:L], lg[:, S2:L], mx[:, 5:6])
        # softmax over 257
        nc.vector.reduce_max(mx[:, 0:1], lg[:, 0:L], axis=mybir.AxisListType.X,
                             negate=True)
        ex = work.tile([16, L], F32R, tag="ex")
        nc.scalar.activation(ex[:, :], lg[:, 0:L], AF.Exp, bias=mx[:, 0:1],
                             accum_out=mx[:, 1:2])
        nc.vector.reciprocal(mx[:, 2:3], mx[:, 1:2])
        # w' = (e_sp + e_m/256) * r ; wm = e_m * r
        wp = work.tile([16, L], F32R, tag="wp")
        nc.vector.tensor_scalar_mul(mx[:, 3:4], ex[:, S2:S2 + 1], 1.0 / S2)
        nc.vector.tensor_scalar(wp[:, 0:S2], ex[:, 0:S2], mx[:, 3:4], mx[:, 2:3],
                                op0=mybir.AluOpType.add,
                                op1=mybir.AluOpType.mult)
        nc.vector.tensor_scalar(wp[:, S2:L], ex[:, S2:L], mx[:, 2:3], None,
                                op0=mybir.AluOpType.mult)
        # transpose w' -> (s-part, h) chunks; third chunk = wm row
        wtp = ps.tile([P, 48], F32R, tag="wt")
        nc.tensor.transpose(wtp[:, 0:16], wp[:, 0:P], ident[:, :])
        nc.tensor.transpose(wtp[:, 16:32], wp[:, P:S2], ident[:, :])
        nc.tensor.transpose(wtp[0:1, 32:48], wp[:, S2:L], ident[:, :])
        for t in range(2):
            nc.vector.tensor_copy(wta[:, t * P + b:(t + 1) * P:8],
                                  wtp[:, t * 16:(t + 1) * 16])
        nc.vector.tensor_copy(wta[0:1, 2 * P + b:3 * P:8], wtp[0:1, 32:48])
        # y_x: stationary xt' tiles, moving w'^T
        yp = ps.tile([P, P], F32, tag="y")
        for j in range(CT):
            for t in range(2):
                nc.tensor.matmul(yp[:, j * 16:(j + 1) * 16],
                                 xtt[:, b, t, j * P:(j + 1) * P],
                                 wta[:, t * P + b:(t + 1) * P:8],
                                 start=(t == 0), stop=(t == 1))
        # scatter y_b into (j, h, b) layout: stride-8 columns for batch b
        nc.vector.tensor_copy(yall[:, b::8], yp[:, :])

    # ---- a0 = blockdiag(W_v) y + (W_v posc) wm + b_v ----
    # the mean-token pos correction y += posc*wm is folded through W_v:
    # a0_h += (W_v[h] posc) * wm[h,b], appended to the psum accumulation.
    a0p = ps1.tile([P, P], F32, tag="seq")
    a0 = acc.tile([P, CT * BPC], BF16)              # (vch-part, (r, b))
    for r in range(CT):
        for j in range(CT):
            nc.tensor.matmul(a0p[:, r * 16:(r + 1) * 16],
                             wvt[:, j, r * P:(r + 1) * P],
                             yall[:, j * P + 2 * r * 8: j * P + 2 * r * 8 + 16],
                             start=(j == 0), stop=False)
        nc.tensor.matmul(a0p[:, r * 16:(r + 1) * 16],
                         wvposc[0:1, r * P:(r + 1) * P],
                         wta[0:1, 2 * P + 2 * r * 8: 2 * P + 2 * r * 8 + 16],
                         start=False, stop=True)
        nc.scalar.activation(a0[0:64, r * 8:(r + 1) * 8],
                             a0p[0:64, r * 16:r * 16 + 8], AF.Identity,
                             bias=small[0:64, 2, r:r + 1])
        nc.scalar.activation(a0[64:P, r * 8:(r + 1) * 8],
                             a0p[64:P, r * 16 + 8:(r + 1) * 16], AF.Identity,
                             bias=small[64:P, 2, r:r + 1])

    # ---- out = w_c a0 + b_c ----
    opf = ps1.tile([P, P], F32, tag="seq")
    op = opf[:, 0:CT * BPC]
    osb = acc.tile([P, CT * BPC], F32)
    for i in range(CT):
        for r in range(CT):
            nc.tensor.matmul(op[:, i * BPC:(i + 1) * BPC],
                             wct[:, r, i * P:(i + 1) * P],
                             a0[:, r * BPC:(r + 1) * BPC],
                             start=(r == 0), stop=(r == CT - 1))
        nc.scalar.activation(osb[:, i * BPC:(i + 1) * BPC],
                             op[:, i * BPC:(i + 1) * BPC], AF.Identity,
                             bias=small[:, 3, i:i + 1])
    if DEBUG:
        for nm, t in [("dxf0", xf0), ("dq0b", q0blk), ("dusb", usb),
                      ("dwta", wta), ("dyall", yall), ("da0", a0)]:
            nc.sync.dma_start(d[nm].ap(), t[:])
    nc.sync.dma_start(d["out"].ap(), osb[:])


DEBUG = False
_CACHE = {}


def _get_nc():
    if "nc" in _CACHE:
        return _CACHE["nc"]
    nc = bacc.Bacc("TRN2", target_bir_lowering=False, debug=False,
                   num_devices=NCORE)
    d = {}
    d["xall"] = nc.dram_tensor("xall", [CT, P, BPC, L], BF16,
                               kind="ExternalInput")
    d["xtp"] = nc.dram_tensor("xtp", [BPC, P, 2, C], BF16,
                              kind="ExternalInput")
    d["small"] = nc.dram_tensor("small", [P, 4, CT], F32,
                                kind="ExternalInput")
    d["wvposc"] = nc.dram_tensor("wvposc", [1, C], BF16, kind="ExternalInput")
    d["wqt"] = nc.dram_tensor("wqt", [CT, P, C], BF16, kind="ExternalInput")
    d["wk"] = nc.dram_tensor("wk", [CT, P, CT, P], BF16, kind="ExternalInput")
    d["wvt"] = nc.dram_tensor("wvt", [CT, P, C], BF16, kind="ExternalInput")
    d["wct"] = nc.dram_tensor("wct", [CT, P, C], BF16, kind="ExternalInput")
    if DEBUG:
        d["dxf0"] = nc.dram_tensor("dxf0", [P, CT * BPC], BF16, kind="ExternalOutput")
        d["dq0b"] = nc.dram_tensor("dq0b", [P, CT * 16], BF16, kind="ExternalOutput")
        d["dusb"] = nc.dram_tensor("dusb", [P, CT * P], BF16, kind="ExternalOutput")
        d["dwta"] = nc.dram_tensor("dwta", [P, 3 * P], BF16, kind="ExternalOutput")
        d["dyall"] = nc.dram_tensor("dyall", [P, CT * P], BF16, kind="ExternalOutput")
        d["da0"] = nc.dram_tensor("da0", [P, CT * BPC], BF16, kind="ExternalOutput")
    d["out"] = nc.dram_tensor("out", [P, CT * BPC], F32, kind="ExternalOutput")
    with tile.TileContext(nc) as tc, ExitStack() as ctx, \
            nc.allow_low_precision(reason="float32r tiles hold f32 bits"):
        _body(ctx, tc, d)
    nc.compile()
    _CACHE["nc"] = nc
    return nc


def _prep_maps(inputs):
    bf16 = ml_dtypes.bfloat16
    xf = inputs["x"].reshape(B, C, S2).astype(np.float32)
    pos = inputs["pos_emb"].astype(np.float32)
    pos_sp = pos[:, 1:]                              # (C, 256)
    posc = pos[:, 0] - pos_sp.mean(axis=1)           # (C,)
    xp = (xf + pos_sp[None]).astype(bf16)            # (B, C, 256)
    xtr = np.ascontiguousarray(
        xp.astype(np.float32).transpose(0, 2, 1)).astype(bf16)  # (B, 256, C)
    posc16 = posc.astype(bf16)

    wqkv = inputs["w_qkv"].astype(np.float32)
    wq, wkm, wv = wqkv[0:C], wqkv[C:2 * C], wqkv[2 * C:]
    wqt = np.ascontiguousarray(wq.T.reshape(CT, P, C)).astype(bf16)
    wkp = np.ascontiguousarray(
        wkm.reshape(CT, P, CT, P).transpose(2, 1, 0, 3)).astype(bf16)
    wvt = np.ascontiguousarray(wv.T.reshape(CT, P, C)).astype(bf16)
    wct = np.ascontiguousarray(
        inputs["w_c"].astype(np.float32).T.reshape(CT, P, C)).astype(bf16)

    bqkv = inputs["b_qkv"].astype(np.float32)
    small = np.empty((P, 4, CT), np.float32)
    small[:, 0] = posc.reshape(CT, P).T
    small[:, 1] = bqkv[0:C].reshape(CT, P).T
    small[:, 2] = bqkv[2 * C:3 * C].reshape(CT, P).T
    small[:, 3] = inputs["b_c"].astype(np.float32).reshape(CT, P).T

    wvposc = (wv @ posc).astype(bf16)
    shared = dict(small=small, wvposc=np.ascontiguousarray(wvposc[None]),
                  wqt=wqt, wk=wkp, wvt=wvt, wct=wct)
    maps = []
    for c in range(NCORE):
        m = dict(shared)
        xc = xp[c * BPC:(c + 1) * BPC]               # (8, C, 256)
        xall = np.empty((CT, P, BPC, L), bf16)
        xall[:, :, :, 0:S2] = xc.reshape(BPC, CT, P, S2).transpose(1, 2, 0, 3)
        xall[:, :, :, S2] = posc16.reshape(CT, P)[:, :, None]
        m["xall"] = xall
        xtc = xtr[c * BPC:(c + 1) * BPC]             # (8, 256, C)
        m["xtp"] = np.ascontiguousarray(
            xtc.reshape(BPC, 2, P, C).transpose(0, 2, 1, 3))
        maps.append(m)
    return maps


def kernel(**inputs) -> np.ndarray:
    nc = _get_nc()
    maps = _prep_maps(inputs)
    res = run_bass_kernel_spmd(nc, maps, list(range(NCORE)))
    outs = []
    for c in range(NCORE):
        arr = res.results[c]["out"].reshape(P, CT, BPC)
        outs.append(arr.transpose(2, 1, 0).reshape(BPC, C))
    return np.concatenate(outs, axis=0).astype(np.float32)


if __name__ == "__main__":
    rng = np.random.default_rng(0)
    ins = {
        "x": rng.standard_normal((B, C, 16, 16), dtype=np.float32),
        "pos_emb": rng.standard_normal((C, L), dtype=np.float32) / 32,
        "w_qkv": rng.standard_normal((3 * C, C), dtype=np.float32) / 32,
        "b_qkv": rng.standard_normal((3 * C,), dtype=np.float32) * 0.1,
        "w_c": rng.standard_normal((C, C), dtype=np.float32) / 32,
        "b_c": rng.standard_normal((C,), dtype=np.float32) * 0.1,
    }
    o = kernel(**ins)
    print("out", o.shape, o.dtype, float(np.abs(o).mean()))


# revision 16
# speedup vs baseline: 1.3193x; 1.0565x over previous
"""AttentionPool2d Trainium2 kernel, 8-core batch-data-parallel.

Math (reference returns only query position 0):
  x' = x.flat + pos_sp (pre-added on host); posc = pos_m - mean(pos_sp)
  xf0 = mean_s(x') + posc    (mean-token input vector; written into x'
                              col 256 so the mean-token logit rides the
                              same 257-wide logits matmul)
  q0 = W_q @ xf0 + b_q                      (the only query needed)
  u_h = W_k_h^T q0_h  (folds W_k into the query; k never materialized)
  l = (1/8) u^T [x' | xf0] ; w = softmax(l) ; w' = w_sp + w_m/256
  y = x'^T w'
  a0_h = W_v_h y_h + (W_v posc) wm_h + b_v  (mean-token pos correction
                                             folded through W_v)
  out = w_c a0 + b_c

DMA: every input host-packed so each partition line is one contiguous
>=2KB descriptor.  Stream order x' -> W_q -> W_k -> xt' -> W_v -> W_c
with compute chasing the stream.  x' slabs issue on the scalar DGE so
descriptor generation overlaps the sync-queue weight stream.
"""
import sys
sys.path.insert(0, "/opt/trn_rl_repo")
import numpy as np
import ml_dtypes
from contextlib import ExitStack

from concourse import bacc, tile, mybir
from concourse import masks
from concourse.bass_utils import run_bass_kernel_spmd

P = 128
B, C, S2, L = 64, 1024, 256, 257
NH, CHD = 16, 64
NCORE, BPC, CT = 8, 8, 8          # cores, batches/core, c-tiles
F32R = mybir.dt.float32r
F32 = mybir.dt.float32
BF16 = mybir.dt.bfloat16
AF = mybir.ActivationFunctionType
SCALE2 = 1.0 / 8.0                 # (1/ch^0.25)^2 folded into u


def _body(ctx: ExitStack, tc, d):
    nc = tc.nc
    const = ctx.enter_context(tc.tile_pool(name="const", bufs=1))
    xres = ctx.enter_context(tc.tile_pool(name="xres", bufs=1))
    xtp = ctx.enter_context(tc.tile_pool(name="xtp", bufs=1))
    wts = ctx.enter_context(tc.tile_pool(name="wts", bufs=1))
    work = ctx.enter_context(tc.tile_pool(name="work", bufs=2))
    acc = ctx.enter_context(tc.tile_pool(name="acc", bufs=1))
    ps = ctx.enter_context(tc.tile_pool(name="ps", bufs=2, space="PSUM"))
    ps1 = ctx.enter_context(tc.tile_pool(name="ps1", bufs=2, space="PSUM"))

    identf = const.tile([16, 16], F32)
    masks.make_identity(nc, identf[:])
    ident = const.tile([16, 16], F32R)
    nc.vector.tensor_copy(ident[:], identf[:, :])

    # ---- DMA issue order = stream order ----
    # x' slabs on the scalar DGE (parallel descriptor-gen with sync queue)
    xt = xres.tile([P, CT, BPC, L], BF16)          # x' (c-part): [p, j, b, 257]
    for j in range(CT):
        nc.scalar.dma_start(xt[:, j, :, 0:S2], d["xall"].ap()[j])
    small = wts.tile([P, 4, CT], F32)              # posc, bq, bv, bc (c-part)
    nc.gpsimd.dma_start(small[:], d["small"].ap())
    wqt = wts.tile([P, CT, C], BF16)               # W_q^T  (c-part, q)
    for h in range(4):
        nc.sync.dma_start(wqt[:, 2 * h:2 * h + 2],
                          d["wqt"].ap()[2 * h:2 * h + 2].rearrange(
                              "j p q -> p j q"))
    wk = wts.tile([P, CT, CT, P], BF16)            # W_k  [kp, j, t, ci]
    for h in range(4):
        nc.sync.dma_start(wk[:, 2 * h:2 * h + 2],
                          d["wk"].ap()[2 * h:2 * h + 2].rearrange(
                              "j p t c -> p j t c"))
    xtt = xtp.tile([P, BPC, 2, C], BF16)           # xt' (s-part): [p, b, t, c]
    for b in range(BPC):
        nc.sync.dma_start(xtt[:, b], d["xtp"].ap()[b])
    wvt = wts.tile([P, CT, C], BF16)               # W_v^T (c-part, vch)
    for h in range(2):
        nc.sync.dma_start(wvt[:, 4 * h:4 * h + 4],
                          d["wvt"].ap()[4 * h:4 * h + 4].rearrange(
                              "j p q -> p j q"))
    wct = wts.tile([P, CT, C], BF16)               # w_c^T (vch-part, o)
    for h in range(2):
        nc.sync.dma_start(wct[:, 4 * h:4 * h + 4],
                          d["wct"].ap()[4 * h:4 * h + 4].rearrange(
                              "j p q -> p j q"))
    wvposc = wts.tile([1, C], BF16)                # W_v @ posc, 1-partition
    nc.sync.dma_start(wvposc[:], d["wvposc"].ap())

    # ---- stage A: means chase x' slabs; xf0 = mean + posc -> x' col 256 ----
    sums = acc.tile([P, CT, BPC], BF16)
    xf0 = acc.tile([P, CT * BPC], BF16)            # (c-part, (j, b))
    for j in range(CT):
        nc.vector.reduce_sum(sums[:, j], xt[:, j, :, 0:S2],
                             axis=mybir.AxisListType.X)
        nc.vector.tensor_scalar(xf0[:, j * BPC:(j + 1) * BPC], sums[:, j],
                                1.0 / S2, small[:, 0, j:j + 1],
                                op0=mybir.AluOpType.mult,
                                op1=mybir.AluOpType.add)
        nc.scalar.activation(xt[:, j, :, S2], xf0[:, j * BPC:(j + 1) * BPC],
                             AF.Copy)

    # ---- stage B: q0 (chases wqt slabs) ----
    q0f = ps1.tile([P, P], F32, tag="seq")
    q0p = q0f[:, 0:CT * BPC]        # (q-part, (i, b))
    for i in range(CT):
        for j in range(CT):
            nc.tensor.matmul(q0p[:, i * BPC:(i + 1) * BPC],
                             wqt[:, j, i * P:(i + 1) * P],
                             xf0[:, j * BPC:(j + 1) * BPC],
                             start=(j == 0), stop=(j == CT - 1))
    # block-diagonal q0 (+bias) for the per-head W_k^T fold
    q0blk = acc.tile([P, CT * 16], BF16)
    nc.vector.memset(q0blk[:], 0.0)
    for i in range(CT):
        nc.vector.tensor_scalar_add(q0blk[0:64, i * 16:i * 16 + 8],
                                    q0p[0:64, i * BPC:i * BPC + 8],
                                    small[0:64, 1, i:i + 1])
        nc.vector.tensor_scalar_add(q0blk[64:P, i * 16 + 8:i * 16 + 16],
                                    q0p[64:P, i * BPC:i * BPC + 8],
                                    small[64:P, 1, i:i + 1])

    # ---- stage C: u = blockdiag(W_k)^T q0, scaled (chases wk slabs) ----
    usb = acc.tile([P, CT * P], BF16)               # (c-part, (j, t2h, b))
    for j in range(CT):
        up = ps1.tile([P, P], F32, tag="seq")
        for t in range(CT):
            nc.tensor.matmul(up[:, t * 16:(t + 1) * 16],
                             wk[:, j, t],
                             q0blk[:, t * 16:(t + 1) * 16])
        nc.vector.tensor_scalar_mul(usb[:, j * P:(j + 1) * P], up[:, :], SCALE2)

    # ---- per-batch: logits (257-wide incl mean token), softmax, y ----
    wta = acc.tile([P, 3 * P], BF16)                # w'^T batched (s-part,(t,h,b))
    yall = acc.tile([P, BPC * P], BF16)             # y (c-part, (b, j, h))
    yv = yall[:, :].rearrange("p (b j h) -> p j h b", b=BPC, j=CT, h=16)
    for b in range(BPC):
        lg = ps.tile([16, L], F32, tag="lg")
        for j in range(CT):
            nc.tensor.matmul(lg[:, 0:L],
                             usb[:, j * P + b: (j + 1) * P: 8],
                             xt[:, j, b, :],
                             start=(j == 0), stop=(j == CT - 1))
        # softmax over 257
        mx = work.tile([16, 4], F32, tag="mx")
        nc.vector.reduce_max(mx[:, 0:1], lg[:, 0:L], axis=mybir.AxisListType.X,
                             negate=True)
        ex = work.tile([16, L], F32R, tag="ex")
        nc.scalar.activation(ex[:, :], lg[:, 0:L], AF.Exp, bias=mx[:, 0:1],
                             accum_out=mx[:, 1:2])
        nc.vector.reciprocal(mx[:, 2:3], mx[:, 1:2])
        # w' = (e_sp + e_m/256) * r ; wm = e_m * r
        wp = work.tile([16, L], F32R, tag="wp")
        nc.vector.tensor_scalar_mul(mx[:, 3:4], ex[:, S2:S2 + 1], 1.0 / S2)
        nc.vector.tensor_scalar(wp[:, 0:S2], ex[:, 0:S2], mx[:, 3:4], mx[:, 2:3],
                                op0=mybir.AluOpType.add,
                                op1=mybir.AluOpType.mult)
        nc.vector.tensor_scalar(wp[:, S2:L], ex[:, S2:L], mx[:, 2:3], None,
                                op0=mybir.AluOpType.mult)
        # transpose w' -> (s-part, h) chunks; third chunk = wm row
        wtp = ps.tile([P, 48], F32R, tag="wt")
        nc.tensor.transpose(wtp[:, 0:16], wp[:, 0:P], ident[:, :])
        nc.tensor.transpose(wtp[:, 16:32], wp[:, P:S2], ident[:, :])
        nc.tensor.transpose(wtp[0:1, 32:48], wp[:, S2:L], ident[:, :])
        nc.vector.tensor_copy(wta[:, b:P:8], wtp[:, 0:16])
        nc.scalar.activation(wta[:, P + b:2 * P:8], wtp[:, 16:32], AF.Copy)
        nc.vector.tensor_copy(wta[0:1, 2 * P + b:3 * P:8], wtp[0:1, 32:48])
        # y_x: stationary xt' tiles, moving w'^T
        yp = ps.tile([P, P], F32, tag="y")
        for j in range(CT):
            for t in range(2):
                nc.tensor.matmul(yp[:, j * 16:(j + 1) * 16],
                                 xtt[:, b, t, j * P:(j + 1) * P],
                                 wta[:, t * P + b:(t + 1) * P:8],
                                 start=(t == 0), stop=(t == 1))
        nc.vector.tensor_copy(yall[:, b * P:(b + 1) * P], yp[:, :])

    # ---- a0 = blockdiag(W_v) y + (W_v posc) wm + b_v ----
    a0 = acc.tile([P, CT * BPC], BF16)              # (vch-part, (r, b))
    for r in range(CT):
        a0p = ps.tile([P, P], F32, tag="y")         # rotate psum banks
        for j in range(CT):
            nc.tensor.matmul(a0p[:, 0:16],
                             wvt[:, j, r * P:(r + 1) * P],
                             yv[:, j, 2 * r:2 * r + 2, :],
                             start=(j == 0), stop=False)
        nc.tensor.matmul(a0p[:, 0:16],
                         wvposc[0:1, r * P:(r + 1) * P],
                         wta[0:1, 2 * P + 2 * r * 8: 2 * P + 2 * r * 8 + 16],
                         start=False, stop=True)
        nc.vector.tensor_scalar_add(a0[0:64, r * BPC:(r + 1) * BPC],
                                    a0p[0:64, 0:BPC],
                                    small[0:64, 2, r:r + 1])
        nc.vector.tensor_scalar_add(a0[64:P, r * BPC:(r + 1) * BPC],
                                    a0p[64:P, BPC:16],
                                    small[64:P, 2, r:r + 1])

    # ---- out = w_c a0 + b_c ----
    osb = acc.tile([P, CT * BPC], F32)
    for i in range(CT):
        op = ps1.tile([P, P], F32, tag="seq")       # rotate psum banks
        for r in range(CT):
            nc.tensor.matmul(op[:, 0:BPC],
                             wct[:, r, i * P:(i + 1) * P],
                             a0[:, r * BPC:(r + 1) * BPC],
                             start=(r == 0), stop=(r == CT - 1))
        nc.scalar.activation(osb[:, i * BPC:(i + 1) * BPC],
                             op[:, 0:BPC], AF.Identity,
                             bias=small[:, 3, i:i + 1])
    nc.sync.dma_start(d["out"].ap(), osb[:])


DEBUG = False
_CACHE = {}


def _get_nc():
    if "nc" in _CACHE:
        return _CACHE["nc"]
    nc = bacc.Bacc("TRN2", target_bir_lowering=False, debug=False,
                   num_devices=NCORE)
    d = {}
    d["xall"] = nc.dram_tensor("xall", [CT, P, BPC, S2], BF16,
                               kind="ExternalInput")
    d["xtp"] = nc.dram_tensor("xtp", [BPC, P, 2, C], BF16,
                              kind="ExternalInput")
    d["small"] = nc.dram_tensor("small", [P, 4, CT], F32,
                                kind="ExternalInput")
    d["wvposc"] = nc.dram_tensor("wvposc", [1, C], BF16, kind="ExternalInput")
    d["wqt"] = nc.dram_tensor("wqt", [CT, P, C], BF16, kind="ExternalInput")
    d["wk"] = nc.dram_tensor("wk", [CT, P, CT, P], BF16, kind="ExternalInput")
    d["wvt"] = nc.dram_tensor("wvt", [CT, P, C], BF16, kind="ExternalInput")
    d["wct"] = nc.dram_tensor("wct", [CT, P, C], BF16, kind="ExternalInput")
    d["out"] = nc.dram_tensor("out", [P, CT * BPC], F32, kind="ExternalOutput")
    with tile.TileContext(nc) as tc, ExitStack() as ctx, \
            nc.allow_low_precision(reason="float32r tiles hold f32 bits"):
        _body(ctx, tc, d)
    nc.compile()
    _CACHE["nc"] = nc
    return nc


def _prep_maps(inputs):
    bf16 = ml_dtypes.bfloat16
    xf = inputs["x"].reshape(B, C, S2).astype(np.float32)
    pos = inputs["pos_emb"].astype(np.float32)
    pos_sp = pos[:, 1:]                              # (C, 256)
    posc = pos[:, 0] - pos_sp.mean(axis=1)           # (C,)
    xp = (xf + pos_sp[None]).astype(bf16)            # (B, C, 256)
    xtr = np.ascontiguousarray(
        xp.astype(np.float32).transpose(0, 2, 1)).astype(bf16)  # (B, 256, C)

    wqkv = inputs["w_qkv"].astype(np.float32)
    wq, wkm, wv = wqkv[0:C], wqkv[C:2 * C], wqkv[2 * C:]
    wqt = np.ascontiguousarray(wq.T.reshape(CT, P, C)).astype(bf16)
    wkp = np.ascontiguousarray(
        wkm.reshape(CT, P, CT, P).transpose(2, 1, 0, 3)).astype(bf16)
    wvt = np.ascontiguousarray(wv.T.reshape(CT, P, C)).astype(bf16)
    wct = np.ascontiguousarray(
        inputs["w_c"].astype(np.float32).T.reshape(CT, P, C)).astype(bf16)

    bqkv = inputs["b_qkv"].astype(np.float32)
    small = np.empty((P, 4, CT), np.float32)
    small[:, 0] = posc.reshape(CT, P).T
    small[:, 1] = bqkv[0:C].reshape(CT, P).T
    small[:, 2] = bqkv[2 * C:3 * C].reshape(CT, P).T
    small[:, 3] = inputs["b_c"].astype(np.float32).reshape(CT, P).T

    wvposc = (wv @ posc).astype(bf16)
    shared = dict(small=small, wvposc=np.ascontiguousarray(wvposc[None]),
                  wqt=wqt, wk=wkp, wvt=wvt, wct=wct)
    maps = []
    for c in range(NCORE):
        m = dict(shared)
        xc = xp[c * BPC:(c + 1) * BPC]               # (8, C, 256)
        m["xall"] = np.ascontiguousarray(
            xc.reshape(BPC, CT, P, S2).transpose(1, 2, 0, 3))
        xtc = xtr[c * BPC:(c + 1) * BPC]             # (8, 256, C)
        m["xtp"] = np.ascontiguousarray(
            xtc.reshape(BPC, 2, P, C).transpose(0, 2, 1, 3))
        maps.append(m)
    return maps


def kernel(**inputs) -> np.ndarray:
    nc = _get_nc()
    maps = _prep_maps(inputs)
    res = run_bass_kernel_spmd(nc, maps, list(range(NCORE)))
    outs = []
    for c in range(NCORE):
        arr = res.results[c]["out"].reshape(P, CT, BPC)
        outs.append(arr.transpose(2, 1, 0).reshape(BPC, C))
    return np.concatenate(outs, axis=0).astype(np.float32)


if __name__ == "__main__":
    rng = np.random.default_rng(0)
    ins = {
        "x": rng.standard_normal((B, C, 16, 16), dtype=np.float32),
        "pos_emb": rng.standard_normal((C, L), dtype=np.float32) / 32,
        "w_qkv": rng.standard_normal((3 * C, C), dtype=np.float32) / 32,
        "b_qkv": rng.standard_normal((3 * C,), dtype=np.float32) * 0.1,
        "w_c": rng.standard_normal((C, C), dtype=np.float32) * 0.1,
        "b_c": rng.standard_normal((C,), dtype=np.float32) * 0.1,
    }
    o = kernel(**ins)
    print("out", o.shape, o.dtype, float(np.abs(o).mean()))
